# revision 23
# baseline (speedup 1.0000x reference)
"""GAT GNN (edge features) Trainium2 kernel — 8-core SPMD.

Sharding: nodes by dst range (6250/core, padded 6272). Per layer: own-slice
matmul (h@Wc with attention-scalar rhs columns) -> bf16 row-table AllGather in
two 25088-row halves (int16-indexable) -> dma_gather edge phase with PSUM
segment-softmax accumulation (numerator matmul + ones-column denominator),
s_dst per slot via onehot^T matmuls, exp/leaky-relu on ScalarE.
"""
import sys

sys.path.insert(0, "/opt/trn_rl_repo")

import numpy as np

NEG_SLOPE = 0.2
EPS = 1e-16
NC = 8
HID = 256
EDGE_DIM = 768
OUT_DIM = 256
N_LAYERS = 6


def make_cfg(n_nodes=50000, n_edges=400000):
    c = {}
    c["N"] = n_nodes
    c["E"] = n_edges
    c["D_CORE"] = n_nodes // NC
    c["D_PAD"] = -(-c["D_CORE"] // 128) * 128
    if (c["D_PAD"] // 128) % 2:
        c["D_PAD"] += 128          # even group count so HALF is 128-aligned
    c["HALF"] = c["D_PAD"] // 2
    c["TBL"] = NC * c["HALF"]
    assert c["TBL"] < 32768
    c["NG"] = c["D_PAD"] // 128
    c["SLOT_W"] = 384              # bf16 slots/row = 768B
    return c


# ---------------- host planner ----------------
def plan(cfg, edge_index):
    src = np.asarray(edge_index[0], np.int64)
    dst = np.asarray(edge_index[1], np.int64)
    DC, HALF, NG = cfg["D_CORE"], cfg["HALF"], cfg["NG"]

    per_core = []
    nbt = 1
    for c in range(NC):
        m = (dst >= c * DC) & (dst < (c + 1) * DC)
        eid = np.nonzero(m)[0]
        es, ed = src[eid], dst[eid] - c * DC
        et = ((es % DC) >= HALF).astype(np.int64)
        erow = (es // DC) * HALF + (es % DC) - et * HALF
        g = ed // 128
        per_core.append((eid, es, ed, et, erow, g))
        cnt = np.zeros((NG, 2), np.int64)
        np.add.at(cnt, (g, et), 1)
        nbt = max(nbt, int(-(-cnt.max() // 128)), 1)
    NBT = nbt
    NBINS = NG * 2 * NBT
    NSLOT = NBINS * 128
    NCALLS = NG * 2

    gidx = np.full((NC, NSLOT), -1, np.int16)
    nreal = np.zeros((NC, NCALLS), np.int32)
    onehT = np.zeros((NC, 128, NSLOT), np.int8)   # [dst_local, slot]
    perm = np.full((NC, NSLOT), -1, np.int64)
    for c in range(NC):
        eid, es, ed, et, erow, g = per_core[c]
        for gg in range(NG):
            for t in (0, 1):
                call = gg * 2 + t
                sel = np.nonzero((g == gg) & (et == t))[0]
                base = ((gg * 2 + t) * NBT) * 128
                if len(sel) == 0:
                    gidx[c, base] = 0
                    nreal[c, call] = 1
                    continue
                slots = base + np.arange(len(sel))
                gidx[c, slots] = erow[sel].astype(np.int16)
                nreal[c, call] = len(sel)
                perm[c, slots] = eid[sel]
                onehT[c, ed[sel] - gg * 128, slots] = 1
    return dict(NBT=NBT, NBINS=NBINS, NSLOT=NSLOT, NCALLS=NCALLS,
                gidx=gidx, nreal=nreal, oneh=onehT, perm=perm)


def wrap_idx16(gidx, call_len):
    """[NSLOT] -> [128, NSLOT//16] with per-call 16-partition wrap."""
    ncalls = gidx.shape[0] // call_len
    blk = gidx.reshape(ncalls, call_len // 16, 16).transpose(2, 0, 1)
    flat = blk.reshape(16, ncalls * (call_len // 16))
    return np.tile(flat, (8, 1))


# ---------------- numpy emulation (plan validation) ----------------
def emulate(cfg, inputs, pl):
    x = np.asarray(inputs["x"], np.float32)
    ea = np.asarray(inputs["edge_attr"], np.float32)
    W1, W2 = np.float32(inputs["W1"]), np.float32(inputs["W2"])
    Wc, We = np.float32(inputs["Wc"]), np.float32(inputs["We"])
    a_s, a_d, a_e = (np.float32(inputs["att_src"]), np.float32(inputs["att_dst"]),
                     np.float32(inputs["att_edge"]))
    bias, W3 = np.float32(inputs["bias"]), np.float32(inputs["W3"])
    DC, DP, HALF, TBL, NG = (cfg["D_CORE"], cfg["D_PAD"], cfg["HALF"],
                             cfg["TBL"], cfg["NG"])
    NSLOT, NBT = pl["NSLOT"], pl["NBT"]

    A = np.zeros((NC, NSLOT, N_LAYERS), np.float32)
    Wal = np.einsum("lkh,lh->lk", We, a_e)
    for c in range(NC):
        real = pl["perm"][c] >= 0
        A[c][real] = ea[pl["perm"][c][real]] @ Wal.T

    h = np.zeros((NC, DP, HID), np.float32)
    for c in range(NC):
        h[c, :DC] = x[c * DC:(c + 1) * DC] @ W1 @ W2

    slot_t = (np.arange(NSLOT) // (128 * NBT)) % 2
    slot_g = np.arange(NSLOT) // (128 * 2 * NBT)
    for l in range(N_LAYERS):
        vs, vd = Wc[l] @ a_s[l], Wc[l] @ a_d[l]
        hmm = h @ Wc[l]
        ssrc, sdst = h @ vs, h @ vd
        T = np.concatenate([hmm[:, :HALF], hmm[:, HALF:]], 0).reshape(2 * TBL, HID)
        S = np.concatenate([ssrc[:, :HALF], ssrc[:, HALF:]], 0).reshape(2 * TBL)
        hnew = np.zeros_like(h)
        for c in range(NC):
            gi = pl["gidx"][c].astype(np.int64)
            valid = gi >= 0
            rowg = slot_t * TBL + gi
            G = np.zeros((NSLOT, HID), np.float32)
            Gs = np.zeros(NSLOT, np.float32)
            G[valid] = T[rowg[valid]]
            Gs[valid] = S[rowg[valid]]
            oh = pl["oneh"][c].astype(np.float32)
            sdsel = np.zeros(NSLOT, np.float32)
            for gg in range(NG):
                sl = slot_g == gg
                sdsel[sl] = oh[:, sl].T @ sdst[c, gg * 128:(gg + 1) * 128]
            alpha = Gs + A[c, :, l] + sdsel
            eac = np.exp(np.where(alpha > 0, alpha, NEG_SLOPE * alpha))
            Se = oh * eac[None, :]
            for gg in range(NG):
                sl = slot_g == gg
                numer = Se[:, sl] @ G[sl]
                denom = Se[:, sl].sum(1)
                hnew[c, gg * 128:(gg + 1) * 128] = numer / (denom + EPS)[:, None]
            hnew[c] += bias[l]
            hnew[c, DC:] = 0.0
        h = hnew

    W3p = W3[:HID] + W3[HID:]
    out = np.maximum(h, 0.0) @ W3p
    return np.concatenate([out[c, :DC] for c in range(NC)], 0)


# ---------------- device kernel ----------------
def build(cfg, pl, oneh_dtype="f8", queues=4, debug_taps=False):
    import concourse.bass as bass
    import concourse.tile as tile
    import concourse.mybir as mybir
    from concourse import bacc

    f32, bf16, i16, i32 = (mybir.dt.float32, mybir.dt.bfloat16,
                           mybir.dt.int16, mybir.dt.int32)
    f8 = mybir.dt.float8e4
    OH_DT = f8 if oneh_dtype == "f8" else bf16
    ACT = mybir.ActivationFunctionType
    ALU = mybir.AluOpType

    DP, HALF, TBL, NG, SW = (cfg["D_PAD"], cfg["HALF"], cfg["TBL"],
                             cfg["NG"], cfg["SLOT_W"])
    NBT, NBINS, NSLOT, NCALLS = (pl["NBT"], pl["NBINS"], pl["NSLOT"],
                                 pl["NCALLS"])
    CL = NBT * 128                    # idxs per gather call
    NKC = HID // 128                  # 2 contraction chunks

    nc = bacc.Bacc(None, target_bir_lowering=False, debug=False,
                   num_swdge_queues=queues)

    # inputs
    xT = nc.dram_tensor("xT", [HID, DP], bf16, kind="ExternalInput")
    eaT = nc.dram_tensor("eaT", [EDGE_DIM, NSLOT], bf16, kind="ExternalInput")
    gidxD = nc.dram_tensor("gidx", [128, NSLOT // 16], i16, kind="ExternalInput")
    nrealD = nc.dram_tensor("nreal", [1, NCALLS], i32, kind="ExternalInput")
    onehD = nc.dram_tensor("oneh", [128, NSLOT], OH_DT, kind="ExternalInput")
    onehTD = nc.dram_tensor("onehT", [128, NSLOT], bf16, kind="ExternalInput")
    WcD = nc.dram_tensor("WcS", [N_LAYERS, HID, HID], f32, kind="ExternalInput")
    WcTD = nc.dram_tensor("WcTS", [N_LAYERS, HID, HID], f32, kind="ExternalInput")
    attD = nc.dram_tensor("attS", [N_LAYERS, 2, HID], f32, kind="ExternalInput")
    WeTD = nc.dram_tensor("WeTS", [N_LAYERS, HID, EDGE_DIM], bf16,
                          kind="ExternalInput")
    aeD = nc.dram_tensor("aeS", [N_LAYERS, HID], f32, kind="ExternalInput")
    W1D = nc.dram_tensor("W1", [HID, HID], f32, kind="ExternalInput")
    W2D = nc.dram_tensor("W2", [HID, HID], f32, kind="ExternalInput")
    W3D = nc.dram_tensor("W3", [2 * HID, OUT_DIM], f32, kind="ExternalInput")
    outD = nc.dram_tensor("out", [DP, OUT_DIM], f32, kind="ExternalOutput")
    dbg = {}
    if debug_taps:
        for nm, shp, dt in [("dbg_hT", [128, 2 * DP], bf16),
                            ("dbg_A", [128, (NSLOT // 128) * 16], bf16),
                            ("dbg_G", [128, NBT * SW], bf16),
                            ("dbg_hx", [128, SW], bf16),
                            ("dbg_sc", [128, 5 * 2 * NBT], f32),
                            ("dbg_Se", [128, 2 * NBT * 128], bf16),
                            ("dbg_gps", [128, HID + 1], f32),
                            ("dbg_sdst", [128, 8], f32),
                            ("dbg_wal", [128, 6 * 6], bf16),
                            ("dbg_A16", [16, min(4096, NSLOT)], bf16)]:
            dbg[nm] = nc.dram_tensor(nm, shp, dt, kind="ExternalOutput")

    # internals
    HrowD = nc.dram_tensor("Hrow", [DP, HID], bf16)
    aginD = nc.dram_tensor("agin", [DP, SW], bf16)
    T0D = nc.dram_tensor("T0", [TBL, SW], bf16, addr_space="Shared")
    T1D = nc.dram_tensor("T1", [TBL, SW], bf16, addr_space="Shared")
    A16D = nc.dram_tensor("A16", [16 * (NSLOT // 128), 128], bf16)

    rg = [list(range(NC))]

    with tile.TileContext(nc) as tc:
        with (
            tc.tile_pool(name="res", bufs=1) as res,
            tc.tile_pool(name="hT", bufs=1) as hTp,
            tc.tile_pool(name="lw", bufs=2) as lw,
            tc.tile_pool(name="hex", bufs=3) as hex_,
            tc.tile_pool(name="gp", bufs=4) as gp,
            tc.tile_pool(name="ohp", bufs=4) as ohp,
            tc.tile_pool(name="sep", bufs=2) as sep,
            tc.tile_pool(name="sc", bufs=2) as scp,
            tc.tile_pool(name="hn", bufs=3) as hnp,
            tc.tile_pool(name="mtp", bufs=2, space="PSUM") as mtp,
            tc.tile_pool(name="gps", bufs=2, space="PSUM") as gpsp,
            tc.tile_pool(name="exp", bufs=2, space="PSUM") as expp,
            tc.tile_pool(name="dns", bufs=2, space="PSUM") as dnsp,
        ):
            gidx_sb = res.tile([128, NSLOT // 16], i16)
            nc.sync.dma_start(gidx_sb[:], gidxD[:])
            nreal_sb = res.tile([1, NCALLS], i32)
            nc.sync.dma_start(nreal_sb[:], nrealD[:])
            oneh_sb = res.tile([128, NSLOT], OH_DT)
            nc.sync.dma_start(oneh_sb[:], onehD[:])
            ones_bf = res.tile([128, 1], bf16)
            nc.vector.memset(ones_bf[:], 1.0)
            A_sb = res.tile([128, 16, NSLOT // 128], bf16)
            hT_sb = hTp.tile([128, NKC, DP], bf16)

            # ---------- A-pass: A16D[l, slot] = (edge_attr @ We_l) @ a_e_l ----
            if True:
                NEC = EDGE_DIM // 128  # 6 contraction chunks over edge_dim
                wal_sb = res.tile([128, NEC, 8], bf16)
                for l in range(N_LAYERS):
                    ae_bf = lw.tile([128, NKC, 1], bf16, tag="ae")
                    ae_f = lw.tile([128, NKC, 1], f32, tag="aef")
                    for kc in range(NKC):
                        nc.sync.dma_start(
                            ae_f[:, kc, :], aeD[l, kc * 128:(kc + 1) * 128, None])
                        nc.vector.tensor_copy(ae_bf[:, kc, :], ae_f[:, kc, :])
                    for ec in range(NEC):
                        wet = lw.tile([128, NKC, 128], bf16, tag="wet")
                        for kc in range(NKC):
                            nc.sync.dma_start(
                                wet[:, kc, :],
                                WeTD[l, kc * 128:(kc + 1) * 128,
                                     ec * 128:(ec + 1) * 128])
                        wps = mtp.tile([128, 258], f32, tag="mt")
                        for kc in range(NKC):
                            nc.tensor.matmul(wps[:, 0:1], wet[:, kc, :],
                                             ae_bf[:, kc, :],
                                             start=(kc == 0), stop=(kc == NKC - 1))
                        nc.vector.tensor_copy(wal_sb[:, ec, l:l + 1], wps[:, 0:1])

                CH = 512
                nchunks = -(-NSLOT // CH)
                for ci in range(nchunks):
                    c0 = ci * CH
                    cw = min(CH, NSLOT - c0)
                    eat = gp.tile([128, NEC, CH], bf16, tag="G")
                    for ec in range(NEC):
                        nc.sync.dma_start(
                            eat[:, ec, :cw],
                            eaT[ec * 128:(ec + 1) * 128, c0:c0 + cw])
                    ap = gpsp.tile([6, CH], f32, tag="gps")
                    for ec in range(NEC):
                        nc.tensor.matmul(ap[:, :cw], wal_sb[:, ec, 0:6],
                                         eat[:, ec, :cw],
                                         start=(ec == 0), stop=(ec == NEC - 1))
                    a16 = lw.tile([16, CH], bf16, tag="a16")
                    nc.vector.memset(a16[:, :], 0.0)
                    nc.vector.tensor_copy(a16[0:6, :cw], ap[:, :cw])
                    nc.sync.dma_start(
                        A16D.rearrange("(j c) p -> j c p", j=16)
                        [:, c0 // 128:(c0 + cw) // 128, :],
                        a16[:, :cw].rearrange("j (c p) -> j c p", p=128))
                if debug_taps:
                    nc.sync.dma_start(
                        dbg["dbg_wal"][:].rearrange("p (a b) -> p a b", b=6),
                        wal_sb[:, :, 0:6])
                nc.sync.dma_start_transpose(
                    A_sb[:].rearrange("p a b -> p (a b)"), A16D[:])
                if debug_taps:
                    nc.sync.dma_start(dbg["dbg_A"][:],
                                      A_sb[:].rearrange("p a b -> p (a b)"))

                # ---------- h0 = x @ W1 @ W2 (kept transposed) ----------
                w_bf = lw.tile([128, NKC, HID], bf16, tag="wmat")
                w_f = lw.tile([128, HID], f32, tag="wmatf")
                for kc in range(NKC):
                    nc.sync.dma_start(w_f[:], W1D[kc * 128:(kc + 1) * 128, :])
                    nc.vector.tensor_copy(w_bf[:, kc, :], w_f[:])
                h1T = hTp.tile([128, NKC, DP], bf16, tag="h1T")
                NCH = -(-DP // 512)
                for mi in range(NKC):
                    for ni in range(NCH):
                        n0 = ni * 512
                        nw = min(512, DP - n0)
                        ps = mtp.tile([128, 512], f32, tag="mt")
                        xt = gp.tile([128, NKC, 512], bf16, tag="G")
                        for kc in range(NKC):
                            nc.sync.dma_start(
                                xt[:, kc, :nw],
                                xT[kc * 128:(kc + 1) * 128, n0:n0 + nw])
                        for kc in range(NKC):
                            nc.tensor.matmul(
                                ps[:, :nw], w_bf[:, kc, mi * 128:(mi + 1) * 128],
                                xt[:, kc, :nw],
                                start=(kc == 0), stop=(kc == NKC - 1))
                        nc.vector.tensor_copy(h1T[:, mi, n0:n0 + nw], ps[:, :nw])
                for kc in range(NKC):
                    w_f = lw.tile([128, HID], f32, tag="wmatf")
                    nc.sync.dma_start(w_f[:], W2D[kc * 128:(kc + 1) * 128, :])
                    nc.vector.tensor_copy(w_bf[:, kc, :], w_f[:])
                for mi in range(NKC):
                    for ni in range(NCH):
                        n0 = ni * 512
                        nw = min(512, DP - n0)
                        ps = mtp.tile([128, 512], f32, tag="mt")
                        for kc in range(NKC):
                            nc.tensor.matmul(
                                ps[:, :nw], w_bf[:, kc, mi * 128:(mi + 1) * 128],
                                h1T[:, kc, n0:n0 + nw],
                                start=(kc == 0), stop=(kc == NKC - 1))
                        nc.vector.tensor_copy(hT_sb[:, mi, n0:n0 + nw], ps[:, :nw])

            if debug_taps:
                nc.sync.dma_start(dbg["dbg_hT"][:],
                                  hT_sb[:].rearrange("p a b -> p (a b)"))
            # ---------- layers ----------
            if True:
                nreal_reg = nc.gpsimd.alloc_register("nreal_reg")
                sdst_bf = res.tile([128, NG], bf16)

                for l in range(N_LAYERS):
                    last = l == N_LAYERS - 1
                    # rhs assembly [128, kc, 258]
                    rhs = lw.tile([128, NKC, 258], bf16, tag="rhs")
                    for kc in range(NKC):
                        w_f = lw.tile([128, HID], f32, tag="wmatf")
                        nc.sync.dma_start(w_f[:],
                                          WcD[l, kc * 128:(kc + 1) * 128, :])
                        nc.vector.tensor_copy(rhs[:, kc, 0:HID], w_f[:])
                    wct = lw.tile([128, NKC, NKC, 128], bf16, tag="wct")
                    att_bf = lw.tile([128, NKC, 2], bf16, tag="att")
                    for kc in range(NKC):
                        for kp in range(NKC):
                            w_f = lw.tile([128, 128], f32, tag="wctf")
                            nc.sync.dma_start(
                                w_f[:],
                                WcTD[l, kp * 128:(kp + 1) * 128,
                                     kc * 128:(kc + 1) * 128])
                            nc.vector.tensor_copy(wct[:, kc, kp, :], w_f[:])
                        a_f = lw.tile([128, 2], f32, tag="attf")
                        nc.sync.dma_start(
                            a_f[:, 0:1], attD[l, 0, kc * 128:(kc + 1) * 128, None])
                        nc.sync.dma_start(
                            a_f[:, 1:2], attD[l, 1, kc * 128:(kc + 1) * 128, None])
                        nc.vector.tensor_copy(att_bf[:, kc, :], a_f[:])
                    for kc in range(NKC):
                        for j in range(2):   # v_s, v_d -> rhs col 256+j
                            vp = mtp.tile([128, 258], f32, tag="mt")
                            for kp in range(NKC):
                                nc.tensor.matmul(vp[:, 0:1], wct[:, kc, kp, :],
                                                 att_bf[:, kp, j:j + 1],
                                                 start=(kp == 0), stop=(kp == NKC - 1))
                            nc.vector.tensor_copy(rhs[:, kc, 256 + j:257 + j],
                                                  vp[:, 0:1])

                    # own-slice matmul + table rows
                    sdst_f = scp.tile([128, NG], f32, tag="sdstf")
                    for g in range(NG):
                        mt = mtp.tile([128, 258], f32, tag="mt")
                        for kc in range(NKC):
                            nc.tensor.matmul(
                                mt[:], hT_sb[:, kc, g * 128:(g + 1) * 128],
                                rhs[:, kc, :],
                                start=(kc == 0), stop=(kc == NKC - 1))
                        hx = hex_.tile([128, SW], bf16, tag="hx")
                        nc.vector.memset(hx[:, HID + 2:SW], 0.0)
                        nc.vector.tensor_copy(hx[:, 0:HID], mt[:, 0:HID])
                        nc.vector.tensor_copy(
                            hx[:, HID:HID + 2].bitcast(f32), mt[:, HID:HID + 1])
                        nc.vector.tensor_copy(sdst_f[:, g:g + 1],
                                              mt[:, HID + 1:HID + 2])
                        if debug_taps and l == 0 and g == 0:
                            nc.sync.dma_start(dbg["dbg_hx"][:], hx[:])
                        nc.sync.dma_start(aginD[g * 128:(g + 1) * 128, :], hx[:])
                        if g == NG // 2 - 1:
                            nc.gpsimd.collective_compute(
                                "AllGather", ALU.bypass, replica_groups=rg,
                                ins=[aginD[0:HALF, :]], outs=[T0D[:]])
                        if g == NG - 1:
                            nc.gpsimd.collective_compute(
                                "AllGather", ALU.bypass, replica_groups=rg,
                                ins=[aginD[HALF:DP, :]], outs=[T1D[:]])
                    nc.vector.tensor_copy(sdst_bf[:], sdst_f[:])
                    if debug_taps and l == 0:
                        nc.sync.dma_start(dbg["dbg_sdst"][:, 0:NG], sdst_f[:])

                    # edge phase
                    for g in range(NG):
                        Gt = [None, None]
                        ex = expp.tile([128, 2 * NBT], f32, tag="ex")
                        for t in (0, 1):
                            call = g * 2 + t
                            G = gp.tile([128, NBT, SW], bf16, tag="G")
                            nc.vector.memset(
                                G[:, :, HID:HID + 2].bitcast(f32), 0.0)
                            nc.gpsimd.reg_load(nreal_reg,
                                               nreal_sb[0:1, call:call + 1])
                            nc.gpsimd.dma_gather(
                                out_ap=G[:], in_ap=(T0D[:] if t == 0 else T1D[:]),
                                idxs_ap=gidx_sb[:, call * (CL // 16):
                                                (call + 1) * (CL // 16)],
                                num_idxs=CL, num_idxs_reg=nreal_reg,
                                elem_size=SW, queue_num=call % queues)
                            Gt[t] = G
                            for b in range(NBT):
                                gbin = call * NBT + b
                                oht = ohp.tile([128, 128], bf16, tag="oht")
                                nc.sync.dma_start(
                                    oht[:], onehTD[:, gbin * 128:(gbin + 1) * 128])
                                nc.tensor.matmul(
                                    ex[:, t * NBT + b:t * NBT + b + 1], oht[:],
                                    sdst_bf[:, g:g + 1], start=True, stop=True)
                        beta = scp.tile([128, 2 * NBT], f32, tag="beta")
                        for t in (0, 1):
                            nc.vector.tensor_tensor(
                                out=beta[:, t * NBT:(t + 1) * NBT],
                                in0=A_sb[:, l,
                                         (g * 2 + t) * NBT:(g * 2 + t + 1) * NBT],
                                in1=Gt[t][:, :, HID:HID + 2].bitcast(f32)[:, :, 0],
                                op=ALU.add)
                        alpha = scp.tile([128, 2 * NBT], f32, tag="alpha")
                        nc.vector.tensor_tensor(out=alpha[:], in0=beta[:],
                                                in1=ex[:], op=ALU.add)
                        e1 = scp.tile([128, 2 * NBT], f32, tag="e1")
                        nc.scalar.activation(e1[:], alpha[:], ACT.Exp)
                        e2 = scp.tile([128, 2 * NBT], f32, tag="e2")
                        nc.scalar.activation(e2[:], alpha[:], ACT.Exp,
                                             scale=NEG_SLOPE)
                        eac = scp.tile([128, 2 * NBT], f32, tag="eac")
                        nc.vector.tensor_tensor(out=eac[:], in0=e1[:], in1=e2[:],
                                                op=ALU.max)
                        Se = sep.tile([128, 2 * NBT, 128], bf16, tag="Se")
                        nc.vector.tensor_tensor(
                            out=Se[:],
                            in0=oneh_sb[:, g * 2 * NBT * 128:
                                        (g + 1) * 2 * NBT * 128]
                                .rearrange("p (a b) -> p a b", b=128),
                            in1=eac[:, :, None].to_broadcast([128, 2 * NBT, 128]),
                            op=ALU.mult)
                        if debug_taps and l == 0 and g == 0:
                            nc.sync.dma_start(
                                dbg["dbg_G"][:],
                                Gt[0][:].rearrange("p a b -> p (a b)"))
                            sc = dbg["dbg_sc"]
                            nc.sync.dma_start(sc[:, 0:2 * NBT], beta[:])
                            nc.sync.dma_start(sc[:, 2 * NBT:4 * NBT], alpha[:])
                            nc.sync.dma_start(sc[:, 4 * NBT:6 * NBT], eac[:])
                            sc2 = scp.tile([128, 2 * NBT], f32, tag="dbg2",
                                           name="sc2")
                            nc.vector.tensor_copy(sc2[:], ex[:])
                            nc.sync.dma_start(sc[:, 6 * NBT:8 * NBT], sc2[:])
                            nc.sync.dma_start(
                                dbg["dbg_Se"][:],
                                Se[:].rearrange("p a b -> p (a b)"))
                        gps = gpsp.tile([128, HID], f32, tag="gps")
                        dns = dnsp.tile([128, 1], f32, tag="dns")
                        for t in (0, 1):
                            for b in range(NBT):
                                i = t * NBT + b
                                nc.tensor.matmul(
                                    gps[:], Se[:, i, :],
                                    Gt[t][:, b, 0:HID],
                                    start=(i == 0), stop=(i == 2 * NBT - 1))
                                nc.tensor.matmul(
                                    dns[:], Se[:, i, :], ones_bf[:],
                                    start=(i == 0), stop=(i == 2 * NBT - 1))
                        # retire
                        dcol = scp.tile([128, 1], f32, tag="dcol")
                        nc.vector.tensor_scalar_add(dcol[:], dns[:], EPS)
                        rcol = scp.tile([128, 1], f32, tag="rcol")
                        nc.vector.reciprocal(rcol[:], dcol[:])
                        hn = hnp.tile([128, HID], bf16, tag="hn")
                        nc.scalar.activation(hn[:], gps[:],
                                             ACT.Relu if last else ACT.Copy,
                                             scale=rcol[:, 0:1])
                        if debug_taps and l == 0 and g == 0:
                            gcp = hnp.tile([128, HID + 1], f32, tag="gcp",
                                           name="gcp")
                            nc.vector.tensor_copy(gcp[:, 0:HID], gps[:])
                            nc.vector.tensor_copy(gcp[:, HID:HID + 1], dns[:])
                            nc.sync.dma_start(dbg["dbg_gps"][:], gcp[:])
                        nc.sync.dma_start(HrowD[g * 128:(g + 1) * 128, :], hn[:])

                    if not last:
                        for kc in range(NKC):
                            nc.sync.dma_start_transpose(
                                hT_sb[:, kc, :],
                                HrowD[:, kc * 128:(kc + 1) * 128])

                # ---------- final: relu(h6) @ (W3a + W3b) ----------
                w3 = lw.tile([128, NKC, OUT_DIM], f32, tag="w3f")
                w3b = lw.tile([128, NKC, OUT_DIM], f32, tag="w3fb")
                w3p = lw.tile([128, NKC, OUT_DIM], bf16, tag="w3p")
                for kc in range(NKC):
                    nc.sync.dma_start(w3[:, kc, :],
                                      W3D[kc * 128:(kc + 1) * 128, :])
                    nc.sync.dma_start(w3b[:, kc, :],
                                      W3D[HID + kc * 128:HID + (kc + 1) * 128, :])
                    nc.vector.tensor_add(w3[:, kc, :], w3[:, kc, :], w3b[:, kc, :])
                    nc.vector.tensor_copy(w3p[:, kc, :], w3[:, kc, :])
                for kc in range(NKC):
                    nc.sync.dma_start_transpose(
                        hT_sb[:, kc, :], HrowD[:, kc * 128:(kc + 1) * 128])
                for g in range(NG):
                    ps = mtp.tile([128, 258], f32, tag="mt")
                    for kc in range(NKC):
                        nc.tensor.matmul(ps[:, 0:OUT_DIM],
                                         hT_sb[:, kc, g * 128:(g + 1) * 128],
                                         w3p[:, kc, :],
                                         start=(kc == 0), stop=(kc == NKC - 1))
                    ot = hnp.tile([128, OUT_DIM], f32, tag="ot")
                    nc.vector.tensor_copy(ot[:], ps[:, 0:OUT_DIM])
                    nc.sync.dma_start(outD[g * 128:(g + 1) * 128, :], ot[:])

    nc.compile()
    return nc


# ---------------- host-side input prep ----------------
def prep_inputs(cfg, pl, inputs, trim=True):
    x = np.asarray(inputs["x"], np.float32)
    ea = np.asarray(inputs["edge_attr"], np.float32)
    DC, DP = cfg["D_CORE"], cfg["D_PAD"]
    NSLOT, NBT = pl["NSLOT"], pl["NBT"]
    ml = __import__("ml_dtypes")
    bf16 = ml.bfloat16
    f8 = ml.float8_e4m3

    WeTS = np.ascontiguousarray(
        np.transpose(np.float32(inputs["We"]), (0, 2, 1))).astype(bf16)
    WcTS = np.ascontiguousarray(
        np.transpose(np.float32(inputs["Wc"]), (0, 2, 1))).astype(np.float32)
    attS = np.stack([np.float32(inputs["att_src"]),
                     np.float32(inputs["att_dst"])], 1)
    common = dict(
        WcS=np.ascontiguousarray(np.float32(inputs["Wc"])),
        WcTS=WcTS, attS=np.ascontiguousarray(attS), WeTS=WeTS,
        aeS=np.ascontiguousarray(np.float32(inputs["att_edge"])),
        W1=np.float32(inputs["W1"]), W2=np.float32(inputs["W2"]),
        W3=np.float32(inputs["W3"]),
    )
    maps = []
    for c in range(NC):
        xs = np.zeros((DP, HID), np.float32)
        xs[:DC] = x[c * DC:(c + 1) * DC]
        eat = np.zeros((NSLOT, EDGE_DIM), np.float32)
        real = pl["perm"][c] >= 0
        eat[real] = ea[pl["perm"][c][real]]
        oh = pl["oneh"][c]
        m = dict(common)
        m["xT"] = np.ascontiguousarray(xs.T).astype(bf16)
        m["eaT"] = np.ascontiguousarray(eat.T).astype(bf16)
        if trim:
            m["gidx"] = wrap_idx16(pl["gidx"][c], NBT * 128)
            m["nreal"] = pl["nreal"][c][None, :].astype(np.int32)
        else:
            m["gidx"] = wrap_idx16(np.maximum(pl["gidx"][c], 0), NBT * 128)
            m["nreal"] = np.full((1, pl["NCALLS"]), NBT * 128, np.int32)
        # plan's oneh is [dst_local, slot] (= onehT, the expand lhsT layout);
        # the Se multiply needs per-bin transposed [slot_p, dst_col] blocks.
        oh_se = np.zeros((128, NSLOT), np.int8)
        for b in range(pl["NBINS"]):
            oh_se[:, b * 128:(b + 1) * 128] = oh[:, b * 128:(b + 1) * 128].T
        m["oneh"] = oh_se.astype(f8)
        m["onehT"] = oh.astype(bf16)
        maps.append(m)
    return maps


_CACHE = {}


def kernel(**inputs) -> np.ndarray:
    from concourse.bass_utils import run_bass_kernel_spmd

    cfg = make_cfg()
    ei = np.asarray(inputs["edge_index"])
    pl = plan(cfg, ei)
    key = ("nc", pl["NBT"])
    if key not in _CACHE:
        _CACHE[key] = build(cfg, pl)
    nc = _CACHE[key]
    maps = prep_inputs(cfg, pl, inputs)
    res = run_bass_kernel_spmd(nc, maps, core_ids=list(range(NC)))
    DC, DP = cfg["D_CORE"], cfg["D_PAD"]
    return np.concatenate([res.results[c]["out"][:DC] for c in range(NC)],
                          0).astype(np.float32)
